# revision 29
# baseline (speedup 1.0000x reference)
"""Fused OOQKV attention-with-generated-transform kernel for Trainium2.

Math (per head h):
  g = gelu(x @ Wg_h + bg_h)            # [T, 64, 64] per-token transform
  q,k,v = x @ W{q,k,v}_h + b           # [T, 64]
  qg[t] = q[t] @ g[t]
  att = softmax(qg @ k^T)              # per batch, no scaling
  out_h = att @ v
Sharding: head-parallel, 1 head per core (8 heads, 8 cores); every core
reads the full (host-pre-transposed) xT.

v2 design — transposed ("wcol-major") phase 1:
  All projections run with the WEIGHT as the PE stationary and xT chunks
  moving, so outputs land [col, token].  Benefits over token-major:
    * biases vary along PARTITIONS -> applied for free as the per-partition
      bias operand of the ACT gelu/identity (saves ~139k PE bias-matmul
      rows of the old layout);
    * kT / qgT emerge in the exact layout phase 2 wants (saves PE
      transposes);
    * ACT writes are contiguous (the old layout's strided gelu write ran
      1237 ns vs 701 contiguous).
  W_aug column tiles: 0 = [Wq|Wq] (q doubled -> partition p of the
  projection holds q[d=p%64], the replicated layout the qg product wants),
  1 = [Wk|Wv], 2..33 = Wg with columns PERMUTED e-major (col = e*64+d), so
  a 128-partition tile holds e in {2j,2j+1} x all 64 d.

  qg[t,e] = sum_d q[t,d]*g[t,(e,d)]: DVE multiplies gT (fp16, 4x DVE
  mode: 2-byte packed SBUF operands) by the replicated q chunk, then a
  0/1 SELECTOR matmul (lhsT = sel_j [128,64], sel_j[p,m] = (m == 2j +
  p//64)) contracts the 64 d-partitions of each e directly into row e of
  a persistent [64,512] qgT PSUM bank, accumulating across all 32 g
  tiles.  fp16 products, f32 PSUM accumulate.

  v is needed token-major for the AV matmul: its transposed projection
  [64, 512] is PE-transposed per 128-token tile (cheap) and augmented
  with a ones column so row 64 of the AV output carries the softmax
  denominator (host divides).

  Phase 2 (per batch, interleaved right after the batch's two token
  chunks finish phase 1): S^T = kT-slice.T @ qgT on PE, exp on ACT (no
  max subtraction; |scores| < 70 so fp32 exp is fine), AV accumulated
  over the batch's 8 token tiles.  exp/gelu live in different ACT tables
  (1.3 us/switch) so phase-2 exps are batched, 2 switches per batch.

The g projection runs fully in fp16 (x cast on gpsimd, Wg sent fp16):
fp16's 10-bit mantissa keeps product noise ~4e-4 while halving PE
LDWEIGHTS cost vs f32r; q/k/v stay f32r.  NOTE: per-chunk fp16 DMAs
from DRAM corrupted their ring tile from the 3rd reuse on (measured),
hence the on-device cast - only load fp16 constants once via DMA.
Measured end-to-end rel err 1.6e-3 vs the 2e-2 gate; 437 us vs the
504 us token-major baseline.
"""

import sys

sys.path.insert(0, "/opt/trn_rl_repo")

import numpy as np

B, N, E, H, D = 4, 1024, 512, 8, 64
T = B * N                 # 4096 flattened tokens
M = 8                     # cores
TC = 512                  # token chunk (moving width)
NCH = T // TC             # 8 token chunks
NKT = E // 128            # 4 contraction tiles
NGT = (D * D) // 128      # 32 g column tiles
NWT = NGT + 2             # + [Wq|Wq] and [Wk|Wv] tiles
NTT = T // 128            # 32 token tiles (for v_sb)
SEL_LAG = 4               # g-groups between prod ready and sel matmul

_cache = {}


def _build():
    if "nc" in _cache:
        return _cache["nc"]
    from contextlib import ExitStack

    import concourse.bass as bass
    import concourse.bacc as bacc
    import concourse.mybir as mybir
    import concourse.tile as tile
    from concourse.masks import make_identity

    F32 = mybir.dt.float32
    F32R = mybir.dt.float32r
    F16 = mybir.dt.float16
    AF = mybir.ActivationFunctionType
    ALU = mybir.AluOpType

    nc = bacc.Bacc(trn_type="TRN2")
    xT_d = nc.dram_tensor("xT", [E, T], F32R, kind="ExternalInput")
    Wqkv_d = nc.dram_tensor("Wqkv", [E, 256], F32R, kind="ExternalInput")
    Wg_d = nc.dram_tensor("Wg", [E, D * D], F16, kind="ExternalInput")
    bias_d = nc.dram_tensor("bias", [128, NWT], F32, kind="ExternalInput")
    sel_d = nc.dram_tensor("sel", [128, NGT * D], F16, kind="ExternalInput")
    outT_d = nc.dram_tensor("outT", [D + 1, T], F32, kind="ExternalOutput")

    with tile.TileContext(nc) as tc, ExitStack() as ctx:
        const = ctx.enter_context(tc.tile_pool(name="const", bufs=1))
        acts = ctx.enter_context(tc.tile_pool(name="acts", bufs=1))

        # --- constants (full-width DMAs: narrow column slices run at
        # ~1/4 DMA bandwidth and made the old startup ~94 us) ---
        Wqkv_sb = []
        Wg_sb = []
        for kt in range(NKT):
            wq = const.tile([128, 256], F32R, tag=f"wqkv{kt}")
            nc.sync.dma_start(wq[:], Wqkv_d[kt * 128:(kt + 1) * 128, :])
            Wqkv_sb.append(wq)
        for kt in range(NKT):
            wg = const.tile([128, D * D], F16, tag=f"wg{kt}")
            Wg_sb.append(wg)
        bias_sb = const.tile([128, NWT], F32)
        nc.sync.dma_start(bias_sb[:], bias_d[:, :])
        sel_sb = const.tile([128, NGT * D], F16)

        def fetch_weights():
            # issued AFTER chunk 0's x DMA so the first projections aren't
            # queued behind 4.5 MB of weights (the sync queue is ordered);
            # sel rides after the first Wg quarter - the gelu stream needs
            # that quarter ~4 us before the first selector matmul needs sel
            HG = (D * D) // 4
            for quar in range(4):
                for kt in range(NKT):
                    nc.sync.dma_start(
                        Wg_sb[kt][:, quar * HG:(quar + 1) * HG],
                        Wg_d[kt * 128:(kt + 1) * 128,
                             quar * HG:(quar + 1) * HG])
                if quar == 0:
                    nc.sync.dma_start(sel_sb[:], sel_d[:, :])
        ident = const.tile([128, 128], F32)
        make_identity(nc, ident[:])
        ident_r = const.tile([128, 128], F32R)
        nc.gpsimd.tensor_copy(ident_r[:], ident[:])

        # --- persistent activations ---
        kT_sb = acts.tile([D, T], F32R)
        qgT_sb = acts.tile([D, T], F32R)
        v_sb = acts.tile([128, NTT, D + 1], F32R)
        ones_nt = const.tile([128, NTT], F32)
        nc.gpsimd.memset(ones_nt[:], 1.0)
        nc.vector.tensor_copy(v_sb[:, :, D], ones_nt[:])

        xpool = ctx.enter_context(tc.tile_pool(name="xp", bufs=2))
        qpool = ctx.enter_context(tc.tile_pool(name="qp", bufs=2))
        gpool = ctx.enter_context(tc.tile_pool(name="gp", bufs=7))
        ppool = ctx.enter_context(tc.tile_pool(name="pp", bufs=7))
        vpool = ctx.enter_context(tc.tile_pool(name="vp", bufs=2))
        espool = ctx.enter_context(tc.tile_pool(name="es", bufs=18))
        outp = ctx.enter_context(tc.tile_pool(name="outp", bufs=3))
        pp_g = ctx.enter_context(tc.tile_pool(name="pg", bufs=5, space="PSUM"))
        pp_qg = ctx.enter_context(tc.tile_pool(name="pqg", bufs=1, space="PSUM"))
        pp_ms = ctx.enter_context(tc.tile_pool(name="pms", bufs=2, space="PSUM"))

        es = {}           # (mt, nch) -> exp(S) tile for the pending batch

        def issue_S(b, block):
            """One quarter of batch b's S+exp: 4 scores tiles + 4 exps.
            block 0,1 = nch 0 (mt 0-3, 4-7); block 2,3 = nch 1."""
            nch, mt0 = block // 2, (block % 2) * 4
            nc0 = b * N + nch * TC
            for mt in range(mt0, mt0 + 4):
                mc0 = b * N + mt * 128
                ps = pp_ms.tile([128, TC], F32, tag="ms", name="s")
                nc.tensor.matmul(ps[:], kT_sb[:, mc0:mc0 + 128],
                                 qgT_sb[:, nc0:nc0 + TC],
                                 start=True, stop=True)
                e_t = espool.tile([128, TC], F32R, tag="es")
                nc.scalar.activation(e_t[:], ps[:], AF.Exp)
                es[(mt, nch)] = e_t

        def issue_AV(b, nch):
            pav = pp_ms.tile([D + 1, TC], F32, tag="ms", name="av")
            for mt in range(8):
                nc.tensor.matmul(pav[:], v_sb[:, b * 8 + mt, :],
                                 es[(mt, nch)][:],
                                 start=(mt == 0), stop=(mt == 7))
            o_t = outp.tile([D + 1, TC], F32, tag="o")
            nc.vector.tensor_copy(o_t[:], pav[:])
            nc.sync.dma_start(
                outT_d[:, b * N + nch * TC:b * N + (nch + 1) * TC], o_t[:])

        xc_next = None

        def fetch_x(c):
            xs = []
            for kt in range(NKT):
                xt = xpool.tile([128, TC], F32R, tag=f"x{kt}")
                nc.sync.dma_start(
                    xt[:], xT_d[kt * 128:(kt + 1) * 128, c * TC:(c + 1) * TC])
                xs.append(xt)
            return xs

        def cast_x(xs, eng=None):
            # fp16 copies of x for the g matmuls, made on the idle gpsimd
            # engine (repeated fp16 DMAs from DRAM corrupted their ring
            # tile from the 3rd reuse on; on-device cast avoids that
            # hardware path entirely).  Chunk 0 casts on DVE: it is idle
            # at startup and ~3x faster per cast, so the first g-group
            # is not gated on the slow gpsimd chain.
            eng = eng or nc.gpsimd
            xg = []
            for kt in range(NKT):
                xt16 = xpool.tile([128, TC], F16, tag=f"xg{kt}")
                eng.tensor_copy(xt16[:], xs[kt][:])
                xg.append(xt16)
            return xg

        xc_next = fetch_x(0)
        fetch_weights()
        xg_next = cast_x(xc_next, eng=nc.vector)
        for c in range(NCH):
            xs, xg = xc_next, xg_next
            if c + 1 < NCH:
                xc_next = fetch_x(c + 1)
            c0 = c * TC

            qrep = None
            sel_q = []          # (j, prod) awaiting selector matmul
            qg_bank = pp_qg.tile([D, TC], F32, tag="qg", name="qg")
            n_sel = 0

            def issue_sel():
                nonlocal n_sel
                j, prod = sel_q.pop(0)
                nc.tensor.matmul(qg_bank[:], sel_sb[:, j * D:(j + 1) * D],
                                 prod[:], start=(n_sel == 0),
                                 stop=(n_sel == NGT - 1))
                n_sel += 1

            for w in range(NWT):
                bank = pp_g.tile([128, TC], F32, tag="g", name=f"bank{w}")
                for kt in range(NKT):
                    if w < 2:
                        lhsT = Wqkv_sb[kt][:, w * 128:(w + 1) * 128]
                        rhs = xs[kt][:]
                    else:
                        lhsT = Wg_sb[kt][:, (w - 2) * 128:(w - 1) * 128]
                        rhs = xg[kt][:]
                    nc.tensor.matmul(bank[:], lhsT, rhs,
                                     start=(kt == 0), stop=(kt == NKT - 1))
                if w == 0:
                    qrep = qpool.tile([128, TC], F16, tag="qrep")
                    nc.scalar.activation(qrep[:], bank[:], AF.Identity,
                                         bias=bias_sb[:, 0:1])
                elif w == 1:
                    # tile 1 = [Wv|Wk]: v on partitions 0:64 (lane-aligned for
                    # the PE transpose), k on 64:128 then DMA-shifted to the
                    # base-0 kT_sb (engines are lane-locked; only DMA/PE move
                    # data across partitions)
                    kvt = vpool.tile([128, TC], F32R, tag="kvt")
                    nc.scalar.activation(kvt[0:D, :], bank[0:D, :],
                                         AF.Identity, bias=bias_sb[0:D, 1:2])
                    nc.scalar.activation(kvt[D:2 * D, :], bank[D:2 * D, :],
                                         AF.Identity, bias=bias_sb[D:2 * D, 1:2])
                    nc.sync.dma_start(kT_sb[:, c0:c0 + TC], kvt[D:2 * D, :])
                    for i in range(TC // 128):
                        ptr = pp_ms.tile([128, D], F32R, tag="ms", name="vtr")
                        nc.tensor.transpose(
                            ptr[:], kvt[0:D, i * 128:(i + 1) * 128],
                            ident_r[0:D, 0:D])
                        nc.vector.tensor_copy(
                            v_sb[:, c * (TC // 128) + i, 0:D], ptr[:])
                elif w == 2 and c + 1 < NCH:
                    xg_next = cast_x(xc_next)
                if w >= 2:
                    j = w - 2
                    gt = gpool.tile([128, TC], F16, tag="gt")
                    nc.scalar.activation(gt[:], bank[:], AF.Gelu,
                                         bias=bias_sb[:, w:w + 1])
                    prod = ppool.tile([128, TC], F16, tag="prod")
                    nc.vector.tensor_tensor(prod[:], gt[:], qrep[:],
                                            op=ALU.mult)
                    sel_q.append((j, prod))
                    if j >= SEL_LAG:
                        issue_sel()
                # previous batch's phase 2 rides inside this chunk in four
                # (4 S + 4 exp) blocks so the ACT gelu stream never blocks
                # longer than the PSUM g-pipeline can coast
                if c >= 2 and c % 2 == 0:
                    pb = c // 2 - 1
                    if w in (4, 12, 20, 28):
                        issue_S(pb, (w - 4) // 8)
                    elif w == 24:
                        issue_AV(pb, 0)
                    elif w == 33:
                        issue_AV(pb, 1)
                elif c == NCH - 1:
                    # batch 3's nch-0 scores only need qgT of chunk 6
                    if w in (8, 16):
                        issue_S(3, (w - 8) // 8)
                    elif w == 28:
                        issue_AV(3, 0)
            while sel_q:
                issue_sel()
            nc.vector.tensor_copy(qgT_sb[:, c0:c0 + TC], qg_bank[:])

            if c == NCH - 1:  # tail: batch 3's nch-1 needs this chunk's qgT
                issue_S(3, 2)
                issue_S(3, 3)
                issue_AV(3, 1)

    nc.compile()
    _cache["nc"] = nc
    return nc


def _make_in_maps(x, Wq, bq, Wk, bk, Wv, bv, Wg, bg):
    x = np.asarray(x, dtype=np.float32)
    xT = np.ascontiguousarray(x.reshape(T, E).T)

    # selector: sel[p, j*64+m] = 1 iff m == 2j + p//64
    sel = np.zeros((128, NGT * D), dtype=np.float32)
    p = np.arange(128)
    for j in range(NGT):
        sel[p, j * D + 2 * j + p // D] = 1.0
    sel = sel.astype(np.float16)

    in_maps = []
    for h in range(M):
        c0 = h * D
        Wqkv = np.empty((E, 256), dtype=np.float32)
        Wqkv[:, 0:D] = Wq[:, c0:c0 + D]
        Wqkv[:, D:128] = Wq[:, c0:c0 + D]
        Wqkv[:, 128:128 + D] = Wv[:, c0:c0 + D]
        Wqkv[:, 128 + D:256] = Wk[:, c0:c0 + D]
        # g columns e-major: col' = e*64 + d <- original d*64 + e
        Wg_h = Wg[:, h * D * D:(h + 1) * D * D].reshape(E, D, D)
        Wg_em = np.ascontiguousarray(
            Wg_h.transpose(0, 2, 1)).reshape(E, D * D)
        Wg_em = Wg_em.astype(np.float16)

        bias = np.empty((128, NWT), dtype=np.float32)
        bias[0:D, 0] = bq[c0:c0 + D]
        bias[D:128, 0] = bq[c0:c0 + D]
        bias[0:D, 1] = bv[c0:c0 + D]
        bias[D:128, 1] = bk[c0:c0 + D]
        bg_h = bg[h * D * D:(h + 1) * D * D].reshape(D, D).T.reshape(D * D)
        bias[:, 2:] = bg_h.reshape(NGT, 128).T

        in_maps.append(dict(xT=xT, Wqkv=Wqkv, Wg=Wg_em,
                            bias=bias, sel=sel))
    return in_maps


def kernel(x, Wq, bq, Wk, bk, Wv, bv, Wg, bg):
    from concourse import bass_utils

    nc = _build()
    in_maps = _make_in_maps(x, Wq, bq, Wk, bk, Wv, bv, Wg, bg)
    res = bass_utils.run_bass_kernel_spmd(nc, in_maps, core_ids=list(range(M)))
    out = np.empty((B, N, H, D), dtype=np.float32)
    for h in range(M):
        oT = res.results[h]["outT"]           # [65, T]
        o = (oT[:D] / oT[D:D + 1]).T          # [T, 64]
        out[:, :, h, :] = o.reshape(B, N, D)
    return out.reshape(B, N, E)
